# revision 9
# baseline (speedup 1.0000x reference)
"""Trainium2 Bass kernel for a 4-layer residual GIN (message passing GNN).

Sharding (8 NeuronCores): dst-shard nodes, core c owns rows [c*PC, (c+1)*PC).
h lives in an AllGather'd HBM table of 8 rank blocks, each [PC+1, H] (one zero
pad row per block). Per layer and per src-range (4 ranges of 25002 table rows,
so int16 gather indices fit): a single-pass dma_gather over a jagged-ELL
column layout (nodes degree-sorted per (core,range); the column-length ladder
is the element-wise max over cores so the SPMD program is uniform), DVE adds
into an accumulator, then dma_scatter_add with unique indices merges into an
HBM agg buffer in canonical node order. Self-loops are added as edges, so
agg == h + sum_neighbors(h). The GIN MLP runs feature-major on PE (weights
stationary), BN batch stats go through a tiny AllReduce, normalize+ReLU is one
fused ACT op, the residual is added node-major from the previous layer's local
h slice, and the result feeds the next AllGather. Graph readout reuses the
gather machinery over graphs in canonical order + an AllReduce.
Note b2 is omitted: BatchNorm of (x + b2) equals BatchNorm of x exactly.
"""

import sys

sys.path.insert(0, "/opt/trn_rl_repo")

import numpy as np

N_NODES = 100000
N_FEAT = 32
HIDDEN = 64
N_LAYERS = 4
N_GRAPHS = 512
BN_EPS = 1e-5
NC = 8

PC = N_NODES // NC            # nodes per core (12500)
NRANGE = 4
RB = N_NODES // NRANGE        # nodes per src-range (25000)
RANK_ROWS = PC + 1            # rank block rows in the AG table (+1 zero row)
RANGE_ROWS = 2 * RANK_ROWS    # 25002
TABLE_ROWS = NC * RANK_ROWS   # 100008
PAD_IDX = PC                  # zero row inside each range block
PCR = ((PC + 127) // 128) * 128   # 12544
NT = PCR // 128               # 98 node tiles
CHUNK = 4096                  # gather slots per dma_gather call


def _r128(x):
    return ((int(x) + 127) // 128) * 128


def _wrap16(stream):
    """int16 [n] -> [128, n/16]: j -> (j%16, j//16), replicated over 8 groups."""
    n = len(stream)
    assert n % 16 == 0
    w = np.ascontiguousarray(stream.reshape(n // 16, 16).T).astype(np.int16)
    return np.tile(w, (8, 1))


def _prep(edge_index, batch):
    """Host-side graph partitioning. Returns (layout, per_core_data)."""
    src = np.asarray(edge_index[0]).astype(np.int64)
    dst = np.asarray(edge_index[1]).astype(np.int64)
    loops = np.arange(N_NODES, dtype=np.int64)
    src = np.concatenate([src, loops])
    dst = np.concatenate([dst, loops])

    core = dst // PC
    rng = src // RB
    dl = dst % PC                                   # canonical row in core
    k_rank = src // PC
    srow = (k_rank % 2) * RANK_ROWS + (src % PC)    # row inside range block

    cnt = np.zeros((NC, NRANGE, PC), np.int32)
    np.add.at(cnt, (core, rng, dl), 1)

    # ladders: Lstar[j][k] = r128(max over cores of #nodes with cnt > k)
    maxd = int(cnt.max())
    ladders = []
    for j in range(NRANGE):
        Ls = np.zeros(maxd, np.int64)
        for c in range(NC):
            h = np.bincount(cnt[c, j], minlength=maxd + 1)
            tail = PC - np.cumsum(h)
            Ls = np.maximum(Ls, tail[:maxd])
        Ls = np.array([_r128(x) for x in Ls], np.int64)
        Ls = Ls[Ls > 0]
        assert len(Ls) > 0
        ladders.append(Ls)

    # chunk schedule: list per range of (n_slots, [(col_k, col_off, len), ...])
    chunk_plan = []
    for j in range(NRANGE):
        frags = []
        for k, L in enumerate(ladders[j]):
            off = 0
            while off < L:
                flen = min(CHUNK, L - off)
                frags.append((k, off, flen))
                off += flen
        chunks, cur, cur_slots = [], [], 0
        for f in frags:
            if cur_slots + f[2] > CHUNK and cur:
                chunks.append((cur_slots, cur))
                cur, cur_slots = [], 0
            cur.append(f)
            cur_slots += f[2]
        if cur:
            chunks.append((cur_slots, cur))
        chunk_plan.append(chunks)

    SC = [int(ladders[j][0]) for j in range(NRANGE)]     # scatter lens (r128'd)
    SG = [int(ladders[j].sum()) for j in range(NRANGE)]  # gather stream lens

    # graph pooling: groups of 128 consecutive graphs; column count per group
    b = np.asarray(batch).astype(np.int64)
    gcnt = np.zeros((NC, N_GRAPHS), np.int64)
    node_core = np.arange(N_NODES) // PC
    np.add.at(gcnt, (node_core, b), 1)
    NGG = N_GRAPHS // 128
    Cg = [max(1, int(gcnt[:, g * 128:(g + 1) * 128].max())) for g in range(NGG)]

    layout = dict(ladders=ladders, chunk_plan=chunk_plan, SC=SC, SG=SG, Cg=Cg)

    per_core = []
    for c in range(NC):
        gidx_parts, sidx_parts = [], []
        for j in range(NRANGE):
            m = (core == c) & (rng == j)
            e_dl = dl[m]
            e_srow = srow[m]
            cn = cnt[c, j]
            order = np.argsort(-cn, kind="stable")
            pos = np.empty(PC, np.int64)
            pos[order] = np.arange(PC)
            eorder = np.argsort(pos[e_dl], kind="stable")
            e_dl = e_dl[eorder]
            e_srow = e_srow[eorder]
            q = pos[e_dl]
            counts_q = np.zeros(PC, np.int64)
            np.add.at(counts_q, q, 1)
            node_off = np.concatenate([[0], np.cumsum(counts_q)[:-1]])
            k_in = np.arange(len(q)) - node_off[q]
            Ls = ladders[j]
            coloff = np.concatenate([[0], np.cumsum(Ls)[:-1]])
            stream = np.full(SG[j], PAD_IDX, np.int64)
            stream[coloff[k_in] + q] = e_srow
            gidx_parts.append(stream)
            sc = np.empty(SC[j], np.int64)
            npos = min(SC[j], PC)
            sc[:npos] = order[:npos]
            if SC[j] > PC:
                sc[PC:SC[j]] = np.arange(PC, SC[j])
            sidx_parts.append(sc)

        gidx = _wrap16(np.concatenate(gidx_parts).astype(np.int16))
        sidx = _wrap16(np.concatenate(sidx_parts).astype(np.int16))

        nb = b[c * PC:(c + 1) * PC]
        gorder = np.argsort(nb, kind="stable")
        counts = gcnt[c]
        goff = np.concatenate([[0], np.cumsum(counts)[:-1]])
        parts = []
        for g in range(NGG):
            st = np.full(Cg[g] * 128, PAD_IDX, np.int64)
            for p in range(128):
                gr = g * 128 + p
                cgr = counts[gr]
                st[np.arange(cgr) * 128 + p] = gorder[goff[gr]:goff[gr] + cgr]
            parts.append(st)
        pgidx = _wrap16(np.concatenate(parts).astype(np.int16))

        per_core.append(dict(gidx=gidx, sidx=sidx, pgidx=pgidx))

    return layout, per_core


def _build(layout):
    import concourse.tile as tile
    import concourse.bacc as bacc
    from concourse import mybir
    from concourse.masks import make_identity

    f32 = mybir.dt.float32
    i16 = mybir.dt.int16
    AF = mybir.ActivationFunctionType
    OP = mybir.AluOpType
    H, H2, F = HIDDEN, 2 * HIDDEN, N_FEAT
    chunk_plan = layout["chunk_plan"]
    SC, SG, Cg = layout["SC"], layout["SG"], layout["Cg"]
    NGG = N_GRAPHS // 128

    nc = bacc.Bacc("TRN2", target_bir_lowering=False, debug=False, num_devices=NC)

    def din(name, shape, dt=f32):
        return nc.dram_tensor(name, list(shape), dt, kind="ExternalInput")

    xs = din("xs", [PC, F])
    Wi = din("Wi", [F, H])
    bi_rep = din("bi_rep", [128, H])
    W1 = din("W1", [N_LAYERS, H, H2])
    b1 = din("b1", [N_LAYERS, H2, 1])
    W2 = din("W2", [N_LAYERS, H2, H])
    gm = din("gm", [N_LAYERS, H, 1])
    bt = din("bt", [N_LAYERS, H, 1])
    Wo1 = din("Wo1", [H, H])
    bo1 = din("bo1", [H, 1])
    Wo2 = din("Wo2", [H, 1])
    bo2 = din("bo2", [1, 1])
    gidx_t = din("gidx", [128, sum(SG) // 16], i16)
    sidx_t = din("sidx", [128, sum(SC) // 16], i16)
    pgidx_t = din("pgidx", [128, (sum(Cg) * 128) // 16], i16)
    out_t = nc.dram_tensor("out", [1, N_GRAPHS], f32, kind="ExternalOutput")

    with tile.TileContext(nc) as tc:
        with tc.tile_pool(name="const", bufs=1) as constp, \
             tc.tile_pool(name="gat", bufs=2) as gatp, \
             tc.tile_pool(name="idx", bufs=2) as idxp, \
             tc.tile_pool(name="acc", bufs=2) as accp, \
             tc.tile_pool(name="big", bufs=1) as bigp, \
             tc.tile_pool(name="work", bufs=3) as workp, \
             tc.tile_pool(name="psum", bufs=2, space="PSUM") as psp, \
             tc.tile_pool(name="dram", bufs=1, space="DRAM") as dramp, \
             tc.tile_pool(name="agdram", bufs=2, space="DRAM") as agdp:

            ident = constp.tile([128, 128], f32)
            make_identity(nc, ident[:])
            zt = constp.tile([128, H], f32)
            nc.vector.memset(zt[:], 0.0)

            Wi_s = constp.tile([F, H], f32)
            nc.sync.dma_start(out=Wi_s[:], in_=Wi[:])
            bi_s = constp.tile([128, H], f32)
            nc.sync.dma_start(out=bi_s[:], in_=bi_rep[:])
            W1_s, W2_s, b1_s, gm_s, bt_s = [], [], [], [], []
            for i in range(N_LAYERS):
                W1_s.append(constp.tile([H, H2], f32, tag=f"w1_{i}", name=f"w1s_{i}"))
                W2_s.append(constp.tile([H2, H], f32, tag=f"w2_{i}", name=f"w2s_{i}"))
                b1_s.append(constp.tile([H2, 1], f32, tag=f"b1_{i}", name=f"b1s_{i}"))
                gm_s.append(constp.tile([H, 1], f32, tag=f"gm_{i}", name=f"gms_{i}"))
                bt_s.append(constp.tile([H, 1], f32, tag=f"bt_{i}", name=f"bts_{i}"))
                nc.sync.dma_start(out=W1_s[i][:], in_=W1[i])
                nc.sync.dma_start(out=W2_s[i][:], in_=W2[i])
                nc.sync.dma_start(out=b1_s[i][:], in_=b1[i])
                nc.sync.dma_start(out=gm_s[i][:], in_=gm[i])
                nc.sync.dma_start(out=bt_s[i][:], in_=bt[i])
            Wo1_s = constp.tile([H, H], f32)
            nc.sync.dma_start(out=Wo1_s[:], in_=Wo1[:])
            bo1_s = constp.tile([H, 1], f32)
            nc.sync.dma_start(out=bo1_s[:], in_=bo1[:])
            Wo2_s = constp.tile([H, 1], f32)
            nc.sync.dma_start(out=Wo2_s[:], in_=Wo2[:])
            bo2_s = constp.tile([1, 1], f32)
            nc.sync.dma_start(out=bo2_s[:], in_=bo2[:])

            agg = dramp.tile([PCR, H], f32)
            zsrc = dramp.tile([PCR, H], f32)
            for t in range(NT):
                nc.sync.dma_start(out=zsrc[t * 128:(t + 1) * 128, :], in_=zt[:])

            m2T = bigp.tile([H, PCR], f32)

            # ---- init: h0 = x @ Wi + b_init, node-major into first AG input ----
            ag_in = agdp.tile([RANK_ROWS, H], f32, tag="agin")

            def init_tile(t, x_nm):
                pxT = psp.tile([F, 128], f32, tag="mmA", name="pxT")
                nc.tensor.transpose(out=pxT[:], in_=x_nm, identity=ident[:])
                xT = workp.tile([F, 128], f32, tag="xT")
                nc.scalar.copy(out=xT[:], in_=pxT[:])
                ph = psp.tile([128, H], f32, tag="mmC", name="ph")
                nc.tensor.matmul(out=ph[:], lhsT=xT[:], rhs=Wi_s[:],
                                 start=True, stop=True)
                h0 = workp.tile([128, H], f32, tag="h0")
                nc.vector.tensor_tensor(out=h0[:], in0=ph[:], in1=bi_s[:],
                                        op=OP.add)
                real = 128 if (t + 1) * 128 <= PC else PC - t * 128
                nc.sync.dma_start(out=ag_in[t * 128:t * 128 + real, :],
                                  in_=h0[:real, :])

            NXT = PC // 128
            XT_CH = 16
            t0 = 0
            while t0 < NXT:
                tn = min(XT_CH, NXT - t0)
                st = gatp.tile([128, XT_CH, F], f32, tag="xld")
                srcap = xs[t0 * 128:(t0 + tn) * 128, :].rearrange(
                    "(t p) f -> p t f", p=128)
                nc.sync.dma_start(out=st[:, :tn, :], in_=srcap)
                for tt in range(tn):
                    init_tile(t0 + tt, st[:, tt, :])
                t0 += tn
            tail = PC - NXT * 128
            if tail:
                stx = gatp.tile([128, 1, F], f32, tag="xld")
                nc.vector.memset(stx[:], 0.0)
                nc.sync.dma_start(out=stx[:tail, 0, :], in_=xs[NXT * 128:, :])
                init_tile(NXT, stx[:, 0, :])
            nc.sync.dma_start(out=ag_in[PC:PC + 1, :], in_=zt[:1, :])

            for li in range(N_LAYERS):
                h_prev = ag_in            # local h slice, node-major [RANK_ROWS, H]
                tbl = agdp.tile([TABLE_ROWS, H], f32, tag="tbl")
                nc.gpsimd.collective_compute(
                    "AllGather", OP.bypass,
                    ins=[h_prev[:].opt()], outs=[tbl[:].opt()],
                    replica_groups=[list(range(NC))])

                # ---- aggregation: gather + ELL column adds + scatter merge ----
                nc.sync.dma_start(out=agg[:], in_=zsrc[:])
                g_done = li and 0
                g_done = 0
                s_done = 0
                for j in range(NRANGE):
                    acc = accp.tile([128, NT, H], f32, tag="acc")
                    tblr = tbl[j * RANGE_ROWS:(j + 1) * RANGE_ROWS, :]
                    for (nsl, frags) in chunk_plan[j]:
                        it = idxp.tile([128, CHUNK // 16], i16, tag="gi")
                        nc.sync.dma_start(
                            out=it[:, :nsl // 16],
                            in_=gidx_t[:, g_done // 16:(g_done + nsl) // 16])
                        gt = gatp.tile([128, CHUNK // 128, H], f32, tag="gt")
                        nc.gpsimd.dma_gather(
                            gt[:, :nsl // 128, :], tblr, it[:, :nsl // 16],
                            nsl, nsl, H, single_packet=False)
                        so = 0
                        for (k, aoff, flen) in frags:
                            a0, a1 = aoff // 128, (aoff + flen) // 128
                            s0, s1 = so // 128, (so + flen) // 128
                            if k == 0:
                                nc.vector.tensor_copy(
                                    out=acc[:, a0:a1, :], in_=gt[:, s0:s1, :])
                            else:
                                nc.vector.tensor_tensor(
                                    out=acc[:, a0:a1, :], in0=acc[:, a0:a1, :],
                                    in1=gt[:, s0:s1, :], op=OP.add)
                            so += flen
                        g_done += nsl
                    sit = idxp.tile([128, _r128(max(SC)) // 16], i16, tag="si")
                    nc.sync.dma_start(
                        out=sit[:, :SC[j] // 16],
                        in_=sidx_t[:, s_done // 16:(s_done + SC[j]) // 16])
                    so2 = 0
                    while so2 < SC[j]:
                        pc_n = min(4096, SC[j] - so2)
                        nc.gpsimd.dma_scatter_add(
                            agg[:], acc[:, so2 // 128:(so2 + pc_n) // 128, :],
                            sit[:, so2 // 16:(so2 + pc_n) // 16],
                            pc_n, pc_n, H, single_packet=False)
                        so2 += pc_n
                    s_done += SC[j]

                # ---- MLP (feature-major) + BN partial stats ----
                sums = workp.tile([H, NT], f32, tag="sums")
                sqs = workp.tile([H, NT], f32, tag="sqs")

                def mlp_tile(t, m_nm, li=li, sums=sums, sqs=sqs):
                    pT = psp.tile([H, 128], f32, tag="mmA", name="pTt")
                    nc.tensor.transpose(out=pT[:], in_=m_nm, identity=ident[:])
                    mT = workp.tile([H, 128], f32, tag="mT")
                    nc.scalar.copy(out=mT[:], in_=pT[:])
                    p1 = psp.tile([H2, 128], f32, tag="mmB", name="p1")
                    nc.tensor.matmul(out=p1[:], lhsT=W1_s[li][:], rhs=mT[:],
                                     start=True, stop=True)
                    r1 = workp.tile([H2, 128], f32, tag="r1")
                    nc.scalar.activation(out=r1[:], in_=p1[:], func=AF.Relu,
                                         bias=b1_s[li][:])
                    p2 = psp.tile([H, 128], f32, tag="mmC", name="p2")
                    nc.tensor.matmul(out=p2[:], lhsT=W2_s[li][:], rhs=r1[:],
                                     start=True, stop=True)
                    real = 128 if (t + 1) * 128 <= PC else PC - t * 128
                    sq = workp.tile([H, 128], f32, tag="sq")
                    nc.scalar.activation(out=m2T[:, t * 128:t * 128 + real],
                                         in_=p2[:, :real], func=AF.Copy,
                                         accum_out=sums[:, t:t + 1])
                    nc.scalar.activation(out=sq[:, :real], in_=p2[:, :real],
                                         func=AF.Square,
                                         accum_out=sqs[:, t:t + 1])
                    if real < 128:
                        nc.scalar.copy(out=m2T[:, t * 128 + real:(t + 1) * 128],
                                       in_=p2[:, real:])

                AG_CH = 16
                t0 = 0
                while t0 < NT:
                    tn = min(AG_CH, NT - t0)
                    st = gatp.tile([128, AG_CH, H], f32, tag="aggld")
                    srcap = agg[t0 * 128:(t0 + tn) * 128, :].rearrange(
                        "(t p) f -> p t f", p=128)
                    nc.sync.dma_start(out=st[:, :tn, :], in_=srcap)
                    for tt in range(tn):
                        mlp_tile(t0 + tt, st[:, tt, :])
                    t0 += tn

                # ---- BN: AllReduce stats, derive scale/shift, normalize ----
                bnp = workp.tile([H, 2], f32, tag="bnp")
                nc.vector.tensor_reduce(out=bnp[:, 0:1], in_=sums[:],
                                        axis=mybir.AxisListType.X, op=OP.add)
                nc.vector.tensor_reduce(out=bnp[:, 1:2], in_=sqs[:],
                                        axis=mybir.AxisListType.X, op=OP.add)
                bn_in = dramp.tile([H, 2], f32, tag="bn_in")
                bn_out = dramp.tile([H, 2], f32, tag="bn_out", addr_space="Shared")
                nc.gpsimd.dma_start(out=bn_in[:], in_=bnp[:])
                nc.gpsimd.collective_compute(
                    "AllReduce", OP.add,
                    ins=[bn_in[:].opt()], outs=[bn_out[:].opt()],
                    replica_groups=[list(range(NC))])
                gst = workp.tile([H, 2], f32, tag="bng")
                nc.gpsimd.dma_start(out=gst[:], in_=bn_out[:])
                mean = workp.tile([H, 1], f32, tag="mean")
                nc.vector.tensor_scalar_mul(out=mean[:], in0=gst[:, 0:1],
                                            scalar1=1.0 / N_NODES)
                ex2 = workp.tile([H, 1], f32, tag="ex2")
                nc.vector.tensor_scalar_mul(out=ex2[:], in0=gst[:, 1:2],
                                            scalar1=1.0 / N_NODES)
                msq = workp.tile([H, 1], f32, tag="msq")
                nc.scalar.square(out=msq[:], in_=mean[:])
                var = workp.tile([H, 1], f32, tag="var")
                nc.vector.tensor_tensor(out=var[:], in0=ex2[:], in1=msq[:],
                                        op=OP.subtract)
                vare = workp.tile([H, 1], f32, tag="vare")
                nc.vector.tensor_scalar_add(out=vare[:], in0=var[:],
                                            scalar1=BN_EPS)
                std = workp.tile([H, 1], f32, tag="std")
                nc.scalar.activation(out=std[:], in_=vare[:], func=AF.Sqrt)
                inv = workp.tile([H, 1], f32, tag="inv")
                nc.vector.reciprocal(out=inv[:], in_=std[:])
                sca = workp.tile([H, 1], f32, tag="sca")
                nc.vector.tensor_tensor(out=sca[:], in0=gm_s[li][:], in1=inv[:],
                                        op=OP.mult)
                msc = workp.tile([H, 1], f32, tag="msc")
                nc.vector.tensor_tensor(out=msc[:], in0=mean[:], in1=sca[:],
                                        op=OP.mult)
                shf = workp.tile([H, 1], f32, tag="shf")
                nc.vector.tensor_tensor(out=shf[:], in0=bt_s[li][:], in1=msc[:],
                                        op=OP.subtract)
                nc.scalar.activation(out=m2T[:], in_=m2T[:], func=AF.Relu,
                                     bias=shf[:], scale=sca[:])

                # ---- residual + write next AG input ----
                ag_next = agdp.tile([RANK_ROWS, H], f32, tag="agin")

                def res_tile(t, hin_nm, h_prev=h_prev, ag_next=ag_next):
                    real = 128 if (t + 1) * 128 <= PC else PC - t * 128
                    pn = psp.tile([128, H], f32, tag="mmA", name="pn")
                    nc.tensor.transpose(out=pn[:],
                                        in_=m2T[:, t * 128:(t + 1) * 128],
                                        identity=ident[:H, :H])
                    hn = workp.tile([128, H], f32, tag="hn")
                    nc.vector.tensor_tensor(out=hn[:real, :], in0=pn[:real, :],
                                            in1=hin_nm[:real, :], op=OP.add)
                    nc.sync.dma_start(out=ag_next[t * 128:t * 128 + real, :],
                                      in_=hn[:real, :])

                t0 = 0
                while t0 < NT:
                    tn = min(AG_CH, NT - t0)
                    rows0 = t0 * 128
                    rows1 = min((t0 + tn) * 128, PC)
                    st = gatp.tile([128, AG_CH, H], f32, tag="aggld")
                    nfull = (rows1 - rows0) // 128
                    if nfull:
                        srcap = h_prev[rows0:rows0 + nfull * 128, :].rearrange(
                            "(t p) f -> p t f", p=128)
                        nc.sync.dma_start(out=st[:, :nfull, :], in_=srcap)
                    rem = (rows1 - rows0) - nfull * 128
                    if rem:
                        nc.sync.dma_start(out=st[:rem, nfull, :],
                                          in_=h_prev[rows0 + nfull * 128:rows1, :])
                    for tt in range(tn):
                        res_tile(t0 + tt, st[:, tt, :])
                    t0 += tn
                nc.sync.dma_start(out=ag_next[PC:PC + 1, :], in_=zt[:1, :])
                ag_in = ag_next

            # ---- graph readout ----
            pg = bigp.tile([128, NGG, H], f32, tag="pg")
            p_done = 0
            for g in range(NGG):
                nsl = Cg[g] * 128
                first = True
                done = 0
                while done < nsl:
                    ch = min(CHUNK, nsl - done)
                    it = idxp.tile([128, CHUNK // 16], i16, tag="gi")
                    nc.sync.dma_start(
                        out=it[:, :ch // 16],
                        in_=pgidx_t[:, (p_done + done) // 16:
                                    (p_done + done + ch) // 16])
                    gt = gatp.tile([128, CHUNK // 128, H], f32, tag="gt")
                    nc.gpsimd.dma_gather(
                        gt[:, :ch // 128, :], ag_in[:], it[:, :ch // 16],
                        ch, ch, H, single_packet=False)
                    red = workp.tile([128, H], f32, tag="pgred")
                    nc.vector.tensor_reduce(
                        out=red[:], in_=gt[:, :ch // 128, :].transpose([0, 2, 1]),
                        axis=mybir.AxisListType.X, op=OP.add)
                    if first:
                        nc.vector.tensor_copy(out=pg[:, g, :], in_=red[:])
                    else:
                        nc.vector.tensor_tensor(out=pg[:, g, :], in0=pg[:, g, :],
                                                in1=red[:], op=OP.add)
                    first = False
                    done += ch
                p_done += nsl

            pg_in = dramp.tile([128, NGG * H], f32, tag="pg_in")
            pg_out = dramp.tile([128, NGG * H], f32, tag="pg_out",
                                addr_space="Shared")
            nc.gpsimd.dma_start(out=pg_in[:],
                                in_=pg[:].rearrange("p g f -> p (g f)"))
            nc.gpsimd.collective_compute(
                "AllReduce", OP.add,
                ins=[pg_in[:].opt()], outs=[pg_out[:].opt()],
                replica_groups=[list(range(NC))])
            pgr = bigp.tile([128, NGG, H], f32, tag="pgr")
            nc.gpsimd.dma_start(out=pgr[:].rearrange("p g f -> p (g f)"),
                                in_=pg_out[:])

            gT = bigp.tile([H, N_GRAPHS], f32, tag="gT")
            for g in range(NGG):
                pt = psp.tile([H, 128], f32, tag="mmA", name="pTt")
                nc.tensor.transpose(out=pt[:], in_=pgr[:, g, :], identity=ident[:])
                nc.scalar.copy(out=gT[:, g * 128:(g + 1) * 128], in_=pt[:])
            pr1 = psp.tile([H, N_GRAPHS], f32, tag="mmB", name="pr1")
            nc.tensor.matmul(out=pr1[:], lhsT=Wo1_s[:], rhs=gT[:],
                             start=True, stop=True)
            r1o = bigp.tile([H, N_GRAPHS], f32, tag="r1o")
            nc.scalar.activation(out=r1o[:], in_=pr1[:], func=AF.Relu,
                                 bias=bo1_s[:])
            pr2 = psp.tile([1, N_GRAPHS], f32, tag="mmC", name="pr2")
            nc.tensor.matmul(out=pr2[:], lhsT=Wo2_s[:], rhs=r1o[:],
                             start=True, stop=True)
            ro = bigp.tile([1, N_GRAPHS], f32, tag="ro")
            nc.vector.tensor_tensor(out=ro[:], in0=pr2[:],
                                    in1=bo2_s[:].to_broadcast([1, N_GRAPHS]),
                                    op=OP.add)
            nc.sync.dma_start(out=out_t[:], in_=ro[:])

    nc.finalize()
    return nc


def _make_in_maps(inputs, per_core):
    x = np.asarray(inputs["x"], np.float32)
    common = dict(
        Wi=np.ascontiguousarray(np.asarray(inputs["W_init"], np.float32)),
        bi_rep=np.tile(np.asarray(inputs["b_init"], np.float32)[None, :],
                       (128, 1)),
        W1=np.ascontiguousarray(np.asarray(inputs["W1"], np.float32)),
        b1=np.ascontiguousarray(np.asarray(inputs["b1"], np.float32)[:, :, None]),
        W2=np.ascontiguousarray(np.asarray(inputs["W2"], np.float32)),
        gm=np.ascontiguousarray(np.asarray(inputs["gamma"], np.float32)[:, :, None]),
        bt=np.ascontiguousarray(np.asarray(inputs["beta"], np.float32)[:, :, None]),
        Wo1=np.ascontiguousarray(np.asarray(inputs["Wo1"], np.float32)),
        bo1=np.ascontiguousarray(np.asarray(inputs["bo1"], np.float32)[:, None]),
        Wo2=np.ascontiguousarray(np.asarray(inputs["Wo2"], np.float32)),
        bo2=np.ascontiguousarray(np.asarray(inputs["bo2"], np.float32)[:, None]),
    )
    in_maps = []
    for c in range(NC):
        m = dict(common)
        m["xs"] = np.ascontiguousarray(x[c * PC:(c + 1) * PC])
        m["gidx"] = per_core[c]["gidx"]
        m["sidx"] = per_core[c]["sidx"]
        m["pgidx"] = per_core[c]["pgidx"]
        in_maps.append(m)
    return in_maps


_CACHE = {}


def _get_program(edge_index, batch):
    key = ("p", int(np.asarray(edge_index).sum()) & 0xFFFFFFFF)
    if key not in _CACHE:
        layout, per_core = _prep(edge_index, batch)
        nc = _build(layout)
        _CACHE[key] = (nc, per_core)
    return _CACHE[key]


def kernel(**inputs):
    from concourse.bass_utils import run_bass_kernel_spmd

    nc, per_core = _get_program(inputs["edge_index"], inputs["batch"])
    in_maps = _make_in_maps(inputs, per_core)
    res = run_bass_kernel_spmd(nc, in_maps, list(range(NC)))
    return np.asarray(res.results[0]["out"]).reshape(-1).astype(np.float32)
